# revision 3
# baseline (speedup 1.0000x reference)
# Trainium2 Bass kernel for nn_BoltzmannMachine: sequential Gibbs sweep over
# N=8192 binary units.
#
# Algorithm (exact, matches the jax reference bit-for-bit on binary states):
#   Work in permuted coordinates: unit a is updated at step a.
#   u <= sigmoid(x/T)  <=>  x >= T*logit(u) = thr  (T > 0), so the device
#   only compares against host-precomputed thresholds; no transcendentals.
#   x = x_base + L @ c with c the fire bits and L the strict lower triangle
#   of the permuted coupling matrix (columns scaled by the free mask).
#   Blocked at B=128: PE (TensorE) accumulates each block's x_base row in
#   PSUM out of 128-column matvec contributions (initial-state columns for
#   future blocks, updated columns u = r + f*c for past blocks); a
#   sequential 2-op-per-unit DVE sweep resolves the block's 128 bits; PE
#   transposes the bit row into a column for downstream blocks' matvecs.
import numpy as np

import concourse.bass as bass  # noqa: F401  (registers engine types)
import concourse.mybir as mybir
from concourse import bacc, tile
from concourse import bass_utils

F32 = mybir.dt.float32
A = mybir.AluOpType

N = 8192
B = 128
K = N // B
N_CORES = 8


def _host_prep(w, initial_state, clamping_degree, T, perm, rand_u):
    T = float(np.asarray(T))
    perm = np.asarray(perm).astype(np.int64)

    wp = np.asarray(w, dtype=np.float32)[perm][:, perm]
    s0p = np.asarray(initial_state, dtype=np.float32)[perm]
    f = (np.asarray(clamping_degree)[perm] == 0).astype(np.float32)
    r = s0p * (1.0 - f)
    uu = np.asarray(rand_u, dtype=np.float64)
    with np.errstate(divide="ignore"):
        thr = (T * (np.log(uu) - np.log1p(-uu))).astype(np.float32)

    WPT = np.ascontiguousarray(wp.T)

    # in-block base contributions (upper-incl-diag @ s0p + strict-lower @ r)
    xb = np.zeros(N, dtype=np.float32)
    for k in range(K):
        blk = slice(k * B, (k + 1) * B)
        Wb = wp[blk, blk]
        xb[blk] = (np.triu(Wb, 0) @ s0p[blk] + np.tril(Wb, -1) @ r[blk]).astype(
            np.float32
        )
    bias = (xb - thr).astype(np.float32)

    # wstrips[k][b, l*B+c] = WPT[l*B+b, k*B+c]: per-dest-block rhs strips,
    # contiguous 32KB per partition for a single DMA.
    tmp = WPT.reshape(K, B, K, B)
    wstrips = np.ascontiguousarray(tmp.transpose(2, 1, 0, 3)).reshape(K, B, N)

    # Triangular-packed diagonal rows + bias, all on partition 0:
    # ldpack[k] = [bias_row(B) | row0(B-1) | row1(B-2) | ... | row126(1)]
    # where row i = strict-lower column i of the f-scaled diag block,
    # i.e. entries L[j,i]*f[i] for j in (i, B).
    PACK = B + (B * (B - 1)) // 2
    ldpack = np.zeros((K, PACK), dtype=np.float32)
    for k in range(K):
        blk = slice(k * B, (k + 1) * B)
        ldT = np.triu(WPT[blk, blk] * f[blk][:, None], 1)
        ldpack[k, :B] = bias[k * B:(k + 1) * B]
        off = B
        for i in range(B - 1):
            ldpack[k, off:off + (B - 1 - i)] = ldT[i, i + 1:]
            off += B - 1 - i


    colsT = lambda v: np.ascontiguousarray(v.reshape(K, B).T)

    dev = {
        "wstrips": wstrips,
        "ldpack": ldpack,
        "s0cols": colsT(s0p),
        "fcols": colsT(f),
        "rcols": colsT(r),
    }
    aux = {"perm": perm, "s0p": s0p, "f": f}
    return dev, aux


def _build():
    nc = bacc.Bacc("TRN2", target_bir_lowering=False, debug=False)

    wstrips_d = nc.dram_tensor("wstrips", [K, B, N], F32, kind="ExternalInput")
    PACK = B + (B * (B - 1)) // 2
    ldpack_d = nc.dram_tensor("ldpack", [K, PACK], F32, kind="ExternalInput")
    s0cols_d = nc.dram_tensor("s0cols", [B, K], F32, kind="ExternalInput")
    fcols_d = nc.dram_tensor("fcols", [B, K], F32, kind="ExternalInput")
    rcols_d = nc.dram_tensor("rcols", [B, K], F32, kind="ExternalInput")
    out_d = nc.dram_tensor("c_out", [1, N], F32, kind="ExternalOutput")

    with tile.TileContext(nc) as tc:
        with (
            tc.tile_pool(name="resident", bufs=1) as res,
            tc.tile_pool(name="wpool", bufs=3) as wpool,
            tc.tile_pool(name="ldpool", bufs=2) as ldpool,
            tc.tile_pool(name="zpool", bufs=2) as zpool,
            tc.tile_pool(name="accp", bufs=3, space="PSUM") as accp,
            tc.tile_pool(name="cpsum", bufs=2, space="PSUM") as cpsum,
        ):
            s0_sb = res.tile([B, K], F32, tag="s0")
            nc.sync.dma_start(out=s0_sb[:, :], in_=s0cols_d.ap())
            f_sb = res.tile([B, K], F32, tag="f")
            nc.sync.dma_start(out=f_sb[:, :], in_=fcols_d.ap())
            r_sb = res.tile([B, K], F32, tag="r")
            nc.sync.dma_start(out=r_sb[:, :], in_=rcols_d.ap())
            u_sb = res.tile([B, K], F32, tag="u")
            c_sb = res.tile([1, N], F32, tag="c")
            ones_sb = res.tile([1, 1], F32, tag="ones")
            nc.vector.memset(ones_sb[:, :], 1.0)

            for k in range(K):
                # accumulate x_base row for block k in PSUM
                wk = wpool.tile([B, N], F32, tag="wk")
                nc.sync.dma_start(out=wk[:, :], in_=wstrips_d.ap()[k])
                ldk = ldpool.tile([1, PACK], F32, tag="ldk")
                nc.sync.dma_start(out=ldk[:, :], in_=ldpack_d.ap()[k:k + 1, :])

                acc = accp.tile([1, B], F32, tag="acc")
                order = (
                    [l for l in range(k + 1, K)]  # s0-side (ready at t=0)
                    + [l for l in range(0, max(k - 1, 0))]  # u-side (early)
                    + ([k - 1] if k >= 1 else [])  # JIT u-side
                )
                for idx, l in enumerate(order):
                    v = s0_sb if l > k else u_sb
                    nc.tensor.matmul(
                        acc[:, :],
                        v[:, l:l + 1],
                        wk[:, l * B:(l + 1) * B],
                        start=(idx == 0),
                        stop=(idx == len(order) - 1),
                    )

                # seed z = acc + bias (row layout, partition 0)
                z = zpool.tile([1, B], F32, tag="z")
                nc.vector.tensor_tensor(
                    out=z[:, :], in0=acc[:, :], in1=ldk[:, 0:B], op=A.add,
                )

                # sequential sweep: 2 DVE ops per unit
                bcell = zpool.tile([1, 1], F32, tag="bcell")
                for i in range(B):
                    nc.vector.tensor_scalar(
                        out=bcell[:, :], in0=z[:, i:i + 1],
                        scalar1=0.0, scalar2=None, op0=A.is_ge,
                    )
                    if i < B - 1:
                        off = B + i * (B - 1) - (i * (i - 1)) // 2
                        nc.vector.scalar_tensor_tensor(
                            out=z[:, i + 1:],
                            in0=ldk[:, off:off + (B - 1 - i)],
                            scalar=bcell[:, :], in1=z[:, i + 1:],
                            op0=A.mult, op1=A.add,
                        )

                # bits row; u column for downstream blocks
                nc.vector.tensor_scalar(
                    out=c_sb[:, k * B:(k + 1) * B], in0=z[:, :],
                    scalar1=0.0, scalar2=None, op0=A.is_ge,
                )
                if k < K - 1:
                    cp = cpsum.tile([B, 1], F32, tag="cp")
                    nc.tensor.matmul(
                        cp[:, :], c_sb[:, k * B:(k + 1) * B], ones_sb[:, :],
                        start=True, stop=True,
                    )
                    nc.vector.scalar_tensor_tensor(
                        out=u_sb[:, k:k + 1], in0=cp[:, :], scalar=f_sb[:, k:k + 1],
                        in1=r_sb[:, k:k + 1], op0=A.mult, op1=A.add,
                    )

            nc.sync.dma_start(out=out_d.ap(), in_=c_sb[:, :])

    nc.compile()
    return nc


_NC_CACHE = None


def _get_nc():
    global _NC_CACHE
    if _NC_CACHE is None:
        _NC_CACHE = _build()
    return _NC_CACHE


def kernel(w, initial_state, clamping_degree, T, perm, rand_u, _trace=False):
    dev, aux = _host_prep(w, initial_state, clamping_degree, T, perm, rand_u)
    nc = _get_nc()
    res = bass_utils.run_bass_kernel_spmd(
        nc,
        [dict(dev) for _ in range(N_CORES)],
        core_ids=list(range(N_CORES)),
        trace=_trace,
    )
    c_bits = np.asarray(res.results[0]["c_out"]).reshape(-1)
    if _trace:
        kernel.last_exec_time_ns = res.exec_time_ns
        kernel.last_results = res

    f, s0p, perm_p = aux["f"], aux["s0p"], aux["perm"]
    final_p = f * c_bits.astype(np.float32) + (1.0 - f) * s0p
    out = np.zeros(N, dtype=np.float32)
    out[perm_p] = final_p
    return out.astype(np.asarray(initial_state).dtype)


# revision 5
# speedup vs baseline: 1.5601x; 1.5601x over previous
# Trainium2 Bass kernel for nn_BoltzmannMachine: sequential Gibbs sweep over
# N=8192 binary units.
#
# Algorithm (exact, matches the jax reference bit-for-bit on binary states):
#   Work in permuted coordinates: unit a is updated at step a.
#   u <= sigmoid(x/T)  <=>  x >= T*logit(u) = thr  (T > 0), so the device
#   only compares against host-precomputed thresholds; no transcendentals.
#   x = x_base + L @ c with c the fire bits and L the strict lower triangle
#   of the permuted coupling matrix (columns scaled by the free mask).
#   Blocked at B=128: PE (TensorE) accumulates each block's x_base row in
#   PSUM out of 128-column matvec contributions (initial-state columns for
#   future blocks, updated columns u = r + f*c for past blocks), with the
#   fp32 weights split into a bf16 hi+lo pair so PE runs at bf16 rate with
#   ~2^-17 relative weight error (x error ~3e-6, far under the minimum
#   compare margin). A sequential DVE sweep resolves each block's 128 bits
#   with ONE fused custom-DVE op per unit: z[j] += L[j,i] * (z[i] >= 0).
#   PE transposes each bit row into a column for downstream block matvecs.
import numpy as np

import concourse.bass as bass  # noqa: F401
import concourse.mybir as mybir
from concourse import bacc, tile
from concourse import bass_utils
from concourse import dve_ops as _dve_ops
from concourse.dve_spec import Spec, Src0, Src1, C0, Zero

F32 = mybir.dt.float32
BF16 = mybir.dt.bfloat16
A = mybir.AluOpType

N_FULL = 8192
B = 128
N_CORES = 8


def _register_gibbs_axpy():
    """Runtime-register the fused sweep op: out = in0 + in1*(s0 >= 0).
    The (C0 + Src1*Zero) form keeps the compare stream-dependent so the
    lowering doesn't hoist it into a latch (IS_GE has no swap complement).
    Src1 (the L row) is always finite, so Src1*Zero == 0 exactly."""
    for op in _dve_ops.OPS:
        if op.name == "GIBBS_AXPY":
            return op
    op = _dve_ops.DveOp(
        "GIBBS_AXPY",
        Spec(
            body=Src0 + Src1 * ((C0 + Src1 * Zero) >= Zero),
            reference=lambda in0, in1, s0, s1, imm2: (
                in0 + in1 * (s0 >= 0.0)
            ).astype(np.float32),
        ),
        subdim=False,
        uops_sha={"v3": "4cebbc5d1fef964b", "v4": "54f17dbd90d668d1"},
    )
    _dve_ops.OPS.append(op)
    _dve_ops.CUSTOM_DVE_SPECS[op.name] = op.spec
    _dve_ops._SUB_OPCODE_FOR_NAME[op.name] = (
        max(_dve_ops._SUB_OPCODE_FOR_NAME.values()) + 1
    )
    return op


GIBBS_AXPY = _register_gibbs_axpy()


def host_prep(w, initial_state, clamping_degree, T, perm, rand_u, N=N_FULL):
    K = N // B
    T = float(np.asarray(T))
    perm = np.asarray(perm).astype(np.int64)

    wp = np.asarray(w, dtype=np.float32)[perm][:, perm]
    s0p = np.asarray(initial_state, dtype=np.float32)[perm]
    f = (np.asarray(clamping_degree)[perm] == 0).astype(np.float32)
    r = s0p * (1.0 - f)
    uu = np.asarray(rand_u, dtype=np.float64)
    with np.errstate(divide="ignore"):
        thr = (T * (np.log(uu) - np.log1p(-uu))).astype(np.float32)

    WPT = np.ascontiguousarray(wp.T)

    # in-block base contributions (upper-incl-diag @ s0p + strict-lower @ r)
    xb = np.zeros(N, dtype=np.float32)
    for k in range(K):
        blk = slice(k * B, (k + 1) * B)
        Wb = wp[blk, blk]
        xb[blk] = (np.triu(Wb, 0) @ s0p[blk] + np.tril(Wb, -1) @ r[blk]).astype(
            np.float32
        )
    bias = (xb - thr).astype(np.float32)

    # wstrips[k][b, l*B+c] = WPT[l*B+b, k*B+c], split into bf16 hi + lo
    tmp = WPT.reshape(K, B, K, B)
    wstrips = np.ascontiguousarray(tmp.transpose(2, 1, 0, 3)).reshape(K, B, N)
    whi = wstrips.astype(mybir.dt.np(BF16))
    wlo = (wstrips - whi.astype(np.float32)).astype(mybir.dt.np(BF16))

    # Triangular-packed diagonal rows + bias, all on partition 0:
    # ldpack[k] = [bias_row(B) | row0(B-1) | ... | row126(1)] where row i
    # holds L[j,i]*f[i] for j in (i, B)  (fp32 — sweep exactness).
    PACK = B + (B * (B - 1)) // 2
    ldpack = np.zeros((K, PACK), dtype=np.float32)
    for k in range(K):
        blk = slice(k * B, (k + 1) * B)
        ldT = np.triu(WPT[blk, blk] * f[blk][:, None], 1)
        ldpack[k, :B] = bias[k * B:(k + 1) * B]
        off = B
        for i in range(B - 1):
            ldpack[k, off:off + (B - 1 - i)] = ldT[i, i + 1:]
            off += B - 1 - i

    colsT = lambda v: np.ascontiguousarray(v.reshape(K, B).T)

    dev = {
        "whi": whi,
        "wlo": wlo,
        "ldpack": ldpack,
        "s0cols": colsT(s0p).astype(mybir.dt.np(BF16)),  # binary: exact
        "fcols": colsT(f),
        "rcols": colsT(r),
    }
    aux = {"perm": perm, "s0p": s0p, "f": f, "N": N}
    return dev, aux


def assemble_output(c_bits, aux):
    f, s0p, perm, N = aux["f"], aux["s0p"], aux["perm"], aux["N"]
    final_p = f * c_bits.astype(np.float32) + (1.0 - f) * s0p
    out = np.zeros(N, dtype=np.float32)
    out[perm] = final_p
    return out


def build(N=N_FULL):
    K = N // B
    PACK = B + (B * (B - 1)) // 2
    nc = bacc.Bacc("TRN2", target_bir_lowering=False, debug=False)

    whi_d = nc.dram_tensor("whi", [K, B, N], BF16, kind="ExternalInput")
    wlo_d = nc.dram_tensor("wlo", [K, B, N], BF16, kind="ExternalInput")
    ldpack_d = nc.dram_tensor("ldpack", [K, PACK], F32, kind="ExternalInput")
    s0cols_d = nc.dram_tensor("s0cols", [B, K], BF16, kind="ExternalInput")
    fcols_d = nc.dram_tensor("fcols", [B, K], F32, kind="ExternalInput")
    rcols_d = nc.dram_tensor("rcols", [B, K], F32, kind="ExternalInput")
    out_d = nc.dram_tensor("c_out", [1, N], F32, kind="ExternalOutput")

    with tile.TileContext(nc) as tc:
        with (
            tc.tile_pool(name="resident", bufs=1) as res,
            tc.tile_pool(name="wpool", bufs=2) as wpool,
            tc.tile_pool(name="ldpool", bufs=2) as ldpool,
            tc.tile_pool(name="zpool", bufs=2) as zpool,
            tc.tile_pool(name="accp", bufs=3, space="PSUM") as accp,
            tc.tile_pool(name="cpsum", bufs=2, space="PSUM") as cpsum,
        ):
            s0_sb = res.tile([B, K], BF16, tag="s0")
            nc.sync.dma_start(out=s0_sb[:, :], in_=s0cols_d.ap())
            f_sb = res.tile([B, K], F32, tag="f")
            nc.sync.dma_start(out=f_sb[:, :], in_=fcols_d.ap())
            r_sb = res.tile([B, K], F32, tag="r")
            nc.sync.dma_start(out=r_sb[:, :], in_=rcols_d.ap())
            u_sb = res.tile([B, K], BF16, tag="u")
            cbf_sb = res.tile([1, N], BF16, tag="cbf")
            ones_sb = res.tile([1, 1], BF16, tag="ones")
            nc.vector.memset(ones_sb[:, :], 1.0)

            for k in range(K):
                whik = wpool.tile([B, N], BF16, tag="whik")
                nc.sync.dma_start(out=whik[:, :], in_=whi_d.ap()[k])
                wlok = wpool.tile([B, N], BF16, tag="wlok")
                nc.sync.dma_start(out=wlok[:, :], in_=wlo_d.ap()[k])
                ldk = ldpool.tile([1, PACK], F32, tag="ldk")
                nc.sync.dma_start(out=ldk[:, :], in_=ldpack_d.ap()[k:k + 1, :])

                acc = accp.tile([1, B], F32, tag="acc")
                order = (
                    [l for l in range(k + 1, K)]  # s0-side (ready at t=0)
                    + [l for l in range(0, max(k - 1, 0))]  # u-side (early)
                    + ([k - 1] if k >= 1 else [])  # JIT u-side
                )
                for idx, l in enumerate(order):
                    v = s0_sb if l > k else u_sb
                    for half, wt in ((0, whik), (1, wlok)):
                        nc.tensor.matmul(
                            acc[:, :],
                            v[:, l:l + 1],
                            wt[:, l * B:(l + 1) * B],
                            start=(idx == 0 and half == 0),
                            stop=(idx == len(order) - 1 and half == 1),
                        )

                # seed z = acc + bias (row layout, partition 0)
                z = zpool.tile([1, B], F32, tag="z")
                nc.vector.tensor_tensor(
                    out=z[:, :], in0=acc[:, :], in1=ldk[:, 0:B], op=A.add,
                )

                # sequential sweep: ONE fused custom op per unit
                for i in range(B - 1):
                    off = B + i * (B - 1) - (i * (i - 1)) // 2
                    nc.vector._custom_dve(
                        GIBBS_AXPY,
                        out=z[:, i + 1:],
                        in0=z[:, i + 1:],
                        in1=ldk[:, off:off + (B - 1 - i)],
                        s0=z[:, i:i + 1],
                    )

                # bits row (bf16 — bits are exact) + u column for later blocks
                nc.vector.tensor_scalar(
                    out=cbf_sb[:, k * B:(k + 1) * B], in0=z[:, :],
                    scalar1=0.0, scalar2=None, op0=A.is_ge,
                )
                if k < K - 1:
                    cp = cpsum.tile([B, 1], F32, tag="cp")
                    nc.tensor.matmul(
                        cp[:, :], cbf_sb[:, k * B:(k + 1) * B], ones_sb[:, :],
                        start=True, stop=True,
                    )
                    nc.vector.scalar_tensor_tensor(
                        out=u_sb[:, k:k + 1], in0=cp[:, :], scalar=f_sb[:, k:k + 1],
                        in1=r_sb[:, k:k + 1], op0=A.mult, op1=A.add,
                    )

            # casting DMA (gpsimd): bf16 bits -> f32 output
            nc.gpsimd.dma_start(out=out_d.ap(), in_=cbf_sb[:, :])

    nc.compile()
    return nc


_NC_CACHE = {}


def _get_nc(N=N_FULL):
    if N not in _NC_CACHE:
        _NC_CACHE[N] = build(N)
    return _NC_CACHE[N]


def kernel(w, initial_state, clamping_degree, T, perm, rand_u, _trace=False):
    dev, aux = host_prep(w, initial_state, clamping_degree, T, perm, rand_u)
    nc = _get_nc()
    res = bass_utils.run_bass_kernel_spmd(
        nc,
        [dict(dev) for _ in range(N_CORES)],
        core_ids=list(range(N_CORES)),
        trace=_trace,
    )
    c_bits = np.asarray(res.results[0]["c_out"]).reshape(-1)
    if _trace:
        kernel.last_exec_time_ns = res.exec_time_ns
        kernel.last_results = res
    return assemble_output(c_bits, aux).astype(np.asarray(initial_state).dtype)
